# revision 22
# baseline (speedup 1.0000x reference)
"""Trainium2 Bass kernel for nn_Decoder (scatter + gaussian conv + CTF filter).

Self-contained: hardcodes shapes/sharding for
  alignment (16,6), shifts (16,2), coords (500000,3), values (500000,),
  ctf (16,256,129) -> out (16,256,256) float32, 8 NeuronCores.

Sharding: pure data-parallel over the batch; each core handles 2 images.
Per core:
  - scatter: per 128-point chunk build both bilinear profile rows in fp8e4
    and accumulate the 256x256 image in PSUM with DoubleRow PE matmuls
    (2 chunks per instruction, both operands fp8e4 -> no power throttle).
    Both profiles are built as fp8 PAIRS packed in uint16 cells (the two
    bilinear taps are always adjacent: floor(c), floor(c)+1), mostly by
    GPSIMD local_scatter whose cost is the dst zero-fill (halved by the
    pair packing); overflow x-groups/half-groups are built densely by ACT
    (Abs then Relu, table preloaded) and DVE (tensor_scalar hat via
    min((c+1)-i, i+(1-c)) clamped by op1=max).  Values v are folded into
    the y-profile so x-profiles are unweighted.  All index/data prep runs
    on DVE as width-112 chains (both images merged into one tile);
    dtype converts use tensor_scalar (ALU speed), never tensor_copy CAST
    (~7ns/elem) and never stride-2 narrow writes (RMW, ~50ns/elem).
    The whole loop is fully unrolled: body boundaries cost ~6us each in
    pipeline drains otherwise.  Engine balance at steady state: GPSIMD
    ~95%, ACT ~100%, DVE ~91%, PE-array ~70% effective.
  - conv+FFT+CTF+iFFT: gaussian conv folded into precomputed DFT
    matrices; the whole linear chain is fp32 matmuls + PE transposes.
"""
import sys
if '/opt/trn_rl_repo' not in sys.path:
    sys.path.insert(0, '/opt/trn_rl_repo')

import numpy as np
import concourse.bass as bass
import concourse.bacc as bacc
import concourse.mybir as mybir
from concourse.tile import TileContext
from concourse.bass_utils import run_bass_kernel_spmd

F8 = mybir.dt.float8e4
F16 = mybir.dt.float16
BF16 = mybir.dt.bfloat16
F32 = mybir.dt.float32
U8 = mybir.dt.uint8
U16 = mybir.dt.uint16
I16 = mybir.dt.int16
I32 = mybir.dt.int32
OP = mybir.AluOpType
ACT = mybir.ActivationFunctionType
PM = mybir.MatmulPerfMode

XSIZE = 256
KX = 129
N_PTS = 500000
B_FULL = 16
N_CORES = 8
IMGS = 2                    # images per core
NCHUNK = 3920               # point chunks per image (128 pts each), padded
NPAD = NCHUNK * 128         # 501760 padded points
G = 8                       # chunks per group / dst tile
NG = 7                      # groups per body per image
BODY_C = G * NG             # 56 chunks per For_i body
N_ITER = NCHUNK // BODY_C   # 70
W2 = IMGS * BODY_C          # 112: merged-image prep width
NCELL = G * 128             # 1024 uint16 pair-cells per dst
HSPLIT = 2                  # hybrid groups: chunks [0,HSPLIT) dense, rest GPS

# per-image x-profile group assignment (group indices 0..6):
#   'D' = DVE dense, 'A' = ACT dense, 'G' = GPSIMD pair-scatter
X_ASSIGN = [
    "AJGGGGG",   # image 0: 1 ACT + half-DVE hybrid + 5 GPS
    "AAHGGGG",   # image 1: 2 ACT + half-ACT hybrid + 4 GPS
]


# ---------------------------------------------------------------- host mats
def _build_mats():
    n = XSIZE
    y = np.arange(n)
    ax = np.arange(5, dtype=np.float64) - 2.0
    g = np.exp(-(ax ** 2) / 2.0)
    gn = g / g.sum()
    Gm = np.zeros((n, n))
    for d in range(-2, 3):
        idx = np.arange(max(0, -d), min(n, n - d))
        Gm[idx, idx + d] = gn[d + 2]
    F = np.exp(-2j * np.pi * np.outer(y, y) / n)
    A = F @ Gm                                               # (256,256)
    Bh = np.exp(-2j * np.pi * np.outer(np.arange(KX), y) / n) @ Gm
    Bm = np.zeros((n, n), complex)
    Bm[:KX] = Bh                                             # kx zero-padded
    IFy = np.exp(+2j * np.pi * np.outer(y, y) / n) / n
    c = np.ones(KX)
    c[1:-1] = 2.0
    EXh = (np.exp(+2j * np.pi * np.outer(y, np.arange(KX)) / n) * c[None, :]) / n
    EX = np.zeros((n, n), complex)
    EX[:, :KX] = EXh

    def lhsT(M):  # (256,256) -> transposed, chunked (2,128,256) f32
        t = np.ascontiguousarray(M.T.reshape(2, 128, 256))
        return t.astype(np.float32)

    mats = {
        "ATr": lhsT(A.real), "ATi": lhsT(A.imag),
        "BrT": lhsT(Bm.real), "BiT": lhsT(Bm.imag), "nBiT": lhsT(-Bm.imag),
        "IFrT": lhsT(IFy.real), "IFiT": lhsT(IFy.imag), "nIFiT": lhsT(-IFy.imag),
        "EXrT": lhsT(EX.real), "nEXiT": lhsT(-EX.imag),
        "ident": np.eye(128, dtype=np.float32),
    }
    return mats


MAT_NAMES = ["ATr", "ATi", "BrT", "BiT", "nBiT", "IFrT", "IFiT", "nIFiT",
             "EXrT", "nEXiT"]

# sc columns (per image, 16 cols): 0-2 x row coeffs, 3-5 y row coeffs,
# 6 y const (128 - sy), 7 x const (128 - sx)
C_A, C_B, C_CY, C_CX = 0, 3, 6, 7


# ---------------------------------------------------------------- bass build
def _build_nc():
    nc = bacc.Bacc()
    xt_in = nc.declare_dram_parameter("xt", [128, NCHUNK], F32, isOutput=False)
    yt_in = nc.declare_dram_parameter("yt", [128, NCHUNK], F32, isOutput=False)
    zt_in = nc.declare_dram_parameter("zt", [128, NCHUNK], F32, isOutput=False)
    vt_in = nc.declare_dram_parameter("vt", [128, NCHUNK], F32, isOutput=False)
    sc_in = nc.declare_dram_parameter("sc", [128, IMGS * 16], F32,
                                      isOutput=False)
    slot_in = nc.declare_dram_parameter("slotB", [128, W2], F32,
                                        isOutput=False)
    slotx_in = nc.declare_dram_parameter("slotBX", [128, W2], F32,
                                         isOutput=False)
    iota_in = nc.declare_dram_parameter("iota", [128, 256], F32,
                                        isOutput=False)
    ctf_in = nc.declare_dram_parameter("ctfT", [IMGS, 256, 256], F32,
                                       isOutput=False)
    mat_in = {m: nc.declare_dram_parameter(m, [2, 128, 256], F32,
                                           isOutput=False)
              for m in MAT_NAMES}
    id_in = nc.declare_dram_parameter("ident", [128, 128], F32, isOutput=False)
    out_d = nc.declare_dram_parameter("out", [IMGS, 256, 256], F32,
                                      isOutput=True)

    with TileContext(nc) as tc:
        with tc.tile_pool(name="inp", bufs=1) as inp, \
             tc.tile_pool(name="mat", bufs=1) as matp, \
             tc.tile_pool(name="tmp", bufs=1) as tmp, \
             tc.tile_pool(name="prep", bufs=3) as prep, \
             tc.tile_pool(name="dstp", bufs=6) as dstp, \
             tc.tile_pool(name="work", bufs=1) as work, \
             tc.tile_pool(name="accp", bufs=1, space="PSUM") as accp, \
             tc.tile_pool(name="eps", bufs=4, space="PSUM") as eps:

            # ---------------- load inputs ----------------
            xt = inp.tile([128, NCHUNK], F32)
            yt = inp.tile([128, NCHUNK], F32)
            zt = inp.tile([128, NCHUNK], F32)
            vt = inp.tile([128, NCHUNK], F32)
            nc.sync.dma_start(xt[:], xt_in[:])
            nc.sync.dma_start(yt[:], yt_in[:])
            nc.sync.dma_start(zt[:], zt_in[:])
            nc.sync.dma_start(vt[:], vt_in[:])

            sc = inp.tile([128, IMGS * 16], F32)
            nc.sync.dma_start(sc[:], sc_in[:])
            slotB = inp.tile([128, W2], F32)
            nc.sync.dma_start(slotB[:], slot_in[:])
            slotBX = inp.tile([128, W2], F32)
            nc.sync.dma_start(slotBX[:], slotx_in[:])
            halfB = inp.tile([128, W2], F32)
            nc.vector.memset(halfB[:], 0.5)
            # iota passes through Abs+Relu (exact on >=0 values) so the ACT
            # function table is loaded before the loop.
            iota_raw = inp.tile([128, 256], F32)
            nc.sync.dma_start(iota_raw[:], iota_in[:])
            iota = inp.tile([128, 256], F32)
            nc.scalar.activation(iota[:], iota_raw[:], ACT.Abs,
                                 bias=0.0, scale=1.0)
            nc.scalar.activation(iota[:], iota[:], ACT.Relu,
                                 bias=0.0, scale=1.0)
            iotab = inp.tile([128, 256], BF16)
            nc.vector.tensor_scalar(iotab[:], iota[:], 1.0, None, op0=OP.mult)

            mats = {}
            for m in MAT_NAMES:
                t0 = matp.tile([128, 256], F32, tag=f"{m}0")
                t1 = matp.tile([128, 256], F32, tag=f"{m}1")
                nc.sync.dma_start(t0[:], mat_in[m][0])
                nc.sync.dma_start(t1[:], mat_in[m][1])
                mats[m] = (t0, t1)
            ident = matp.tile([128, 128], F32)
            nc.sync.dma_start(ident[:], id_in[:])
            ctfs = []
            for b in range(IMGS):
                c0 = matp.tile([128, 256], F32, tag=f"ctf{b}0")
                c1 = matp.tile([128, 256], F32, tag=f"ctf{b}1")
                nc.sync.dma_start(c0[:], ctf_in[b, 0:128, :])
                nc.sync.dma_start(c1[:], ctf_in[b, 128:256, :])
                ctfs.append((c0, c1))

            zero16 = inp.tile([128, 256], F16)
            nc.vector.memset(zero16[:], 0.0)

            # GPSIMD local_scatter ucode preload
            dum_idx = inp.tile([128, 2], I16)
            nc.vector.memset(dum_idx[:], -1.0)
            dum_dat = inp.tile([128, 2], U16)
            nc.vector.memset(dum_dat[:], 0.0)
            dum_dst = inp.tile([128, 2], U16)
            nc.gpsimd.local_scatter(dum_dst[:], dum_dat[:], dum_idx[:],
                                    channels=128, num_elems=2, num_idxs=2)

            # ---------------- PSUM accumulators ----------------
            acc = [[accp.tile([128, 256], F32, tag=f"acc{b}{h}",
                               name=f"acc_{b}_{h}")
                    for h in range(2)] for b in range(IMGS)]
            for b in range(IMGS):
                for h in range(2):
                    nc.tensor.matmul(acc[b][h][:], zero16[:, 0:128],
                                     zero16[:], start=True, stop=False,
                                     skip_group_check=True)

            # ---------------- main scatter loop ----------------
            def proj(dst, base, cc0, cc1, cc2, last_scalar):
                """dst = xt*c0 + yt*c1 + zt*c2 + scalar over BODY_C chunks."""
                t0 = tmp.tile([128, BODY_C], F32, tag="p_t0")
                nc.vector.tensor_scalar(
                    t0[:], xt[:, bass.DynSlice(base, BODY_C)], cc0,
                    last_scalar, op0=OP.mult, op1=OP.add)
                t1 = tmp.tile([128, BODY_C], F32, tag="p_t1")
                nc.vector.scalar_tensor_tensor(
                    t1[:], yt[:, bass.DynSlice(base, BODY_C)], cc1, t0[:],
                    op0=OP.mult, op1=OP.add)
                nc.vector.scalar_tensor_tensor(
                    dst, zt[:, bass.DynSlice(base, BODY_C)], cc2, t1[:],
                    op0=OP.mult, op1=OP.add)

            def emit_body(base):
                vcurB = prep.tile([128, W2], F32, tag="vcurB")
                nc.vector.tensor_scalar(vcurB[:, 0:BODY_C],
                                        vt[:, bass.DynSlice(base, BODY_C)],
                                        1.0, None, op0=OP.mult)
                nc.vector.tensor_scalar(vcurB[:, BODY_C:W2],
                                        vt[:, bass.DynSlice(base, BODY_C)],
                                        1.0, None, op0=OP.mult)

                cyoB = tmp.tile([128, W2], F32, tag="cyoB")
                cxoB = tmp.tile([128, W2], F32, tag="cxoB")
                for b in range(IMGS):
                    o = 16 * b
                    sl = slice(BODY_C * b, BODY_C * (b + 1))
                    proj(cxoB[:, sl], base, sc[:, o + C_A:o + C_A + 1],
                         sc[:, o + C_A + 1:o + C_A + 2],
                         sc[:, o + C_A + 2:o + C_A + 3],
                         sc[:, o + C_CX:o + C_CX + 1])
                    proj(cyoB[:, sl], base, sc[:, o + C_B:o + C_B + 1],
                         sc[:, o + C_B + 1:o + C_B + 2],
                         sc[:, o + C_B + 2:o + C_B + 3],
                         sc[:, o + C_CY:o + C_CY + 1])

                # ---- interleaved y/x pair-build chains (width 112) ----
                def T(nm, dt=F32, w=W2):
                    return tmp.tile([128, w], dt, tag=nm, name=nm)

                hcy, hcx = T("hcy"), T("hcx")
                nc.vector.tensor_scalar(hcy[:], cyoB[:], 0.5, None, op0=OP.mult)
                nc.vector.tensor_scalar(hcx[:], cxoB[:], 0.5, None, op0=OP.mult)
                iiy, iix = T("iiy", I32), T("iix", I32)
                nc.vector.tensor_scalar(iiy[:], hcy[:], 1.0, None, op0=OP.mult)
                nc.vector.tensor_scalar(iix[:], hcx[:], 1.0, None, op0=OP.mult)
                ddy, ddx = T("ddy"), T("ddx")
                nc.vector.tensor_scalar(ddy[:], iiy[:], 1.0, None, op0=OP.mult)
                nc.vector.tensor_scalar(ddx[:], iix[:], 1.0, None, op0=OP.mult)
                gty, gtx = T("gty"), T("gtx")
                nc.vector.tensor_tensor(gty[:], ddy[:], hcy[:], op=OP.is_gt)
                nc.vector.tensor_tensor(gtx[:], ddx[:], hcx[:], op=OP.is_gt)
                hfly, hflx = T("hfly"), T("hflx")
                nc.vector.tensor_tensor(hfly[:], ddy[:], gty[:], op=OP.subtract)
                nc.vector.tensor_tensor(hflx[:], ddx[:], gtx[:], op=OP.subtract)
                ry, rx = T("ry"), T("rx")
                nc.vector.tensor_tensor(ry[:], hcy[:], hfly[:], op=OP.subtract)
                nc.vector.tensor_tensor(rx[:], hcx[:], hflx[:], op=OP.subtract)
                my, mx = T("my"), T("mx")
                nc.vector.tensor_tensor(my[:], ry[:], halfB[:], op=OP.is_ge)
                nc.vector.tensor_tensor(mx[:], rx[:], halfB[:], op=OP.is_ge)
                fyB, fxB = T("fyB"), T("fxB")
                nc.vector.scalar_tensor_tensor(fyB[:], ry[:], 2.0, my[:],
                                               op0=OP.mult, op1=OP.subtract)
                nc.vector.scalar_tensor_tensor(fxB[:], rx[:], 2.0, mx[:],
                                               op0=OP.mult, op1=OP.subtract)
                w1v = T("w1v")
                nc.vector.tensor_tensor(w1v[:], vcurB[:], fyB[:], op=OP.mult)
                w0x = T("w0x")
                nc.vector.tensor_scalar(w0x[:], fxB[:], -1.0, 1.0,
                                        op0=OP.mult, op1=OP.add)
                w0v = T("w0v")
                nc.vector.tensor_tensor(w0v[:], vcurB[:], w1v[:], op=OP.subtract)
                c8y = T("c8y", F8, 2 * W2)
                c8x = T("c8x", F8, 2 * W2)
                nc.vector.tensor_scalar(c8x[:, 0:W2], w0x[:], 1.0, None,
                                        op0=OP.mult)
                nc.vector.tensor_scalar(c8x[:, W2:2 * W2], fxB[:], 1.0, None,
                                        op0=OP.mult)
                nc.vector.tensor_scalar(c8y[:, 0:W2], w0v[:], 1.0, None,
                                        op0=OP.mult)
                nc.vector.tensor_scalar(c8y[:, W2:2 * W2], w1v[:], 1.0, None,
                                        op0=OP.mult)
                bfy, bfx = T("bfy", F32, 2 * W2), T("bfx", F32, 2 * W2)
                nc.vector.tensor_scalar(bfy[:], c8y[:].bitcast(U8), 1.0, None,
                                        op0=OP.mult)
                nc.vector.tensor_scalar(bfx[:], c8x[:].bitcast(U8), 1.0, None,
                                        op0=OP.mult)
                qvy, qvx = T("qvy"), T("qvx")
                nc.vector.tensor_scalar(qvy[:], my[:], 255.0, 1.0,
                                        op0=OP.mult, op1=OP.add)
                nc.vector.tensor_scalar(qvx[:], mx[:], 255.0, 1.0,
                                        op0=OP.mult, op1=OP.add)
                p1y, p1x = T("p1y"), T("p1x")
                nc.vector.tensor_tensor(p1y[:], bfy[:, 0:W2], qvy[:], op=OP.mult)
                nc.vector.tensor_tensor(p1x[:], bfx[:, 0:W2], qvx[:], op=OP.mult)
                eyt, ext = T("eyt"), T("ext")
                nc.vector.tensor_scalar(eyt[:], my[:], -1.0, 1.0,
                                        op0=OP.mult, op1=OP.add)
                nc.vector.tensor_scalar(ext[:], mx[:], -1.0, 1.0,
                                        op0=OP.mult, op1=OP.add)
                rby, rbx = T("rby"), T("rbx")
                nc.vector.tensor_tensor(rby[:], bfy[:, W2:2 * W2], eyt[:],
                                        op=OP.mult)
                nc.vector.tensor_tensor(rbx[:], bfx[:, W2:2 * W2], ext[:],
                                        op=OP.mult)
                d0y, d0x = T("d0y"), T("d0x")
                nc.vector.scalar_tensor_tensor(d0y[:], rby[:], 256.0, p1y[:],
                                               op0=OP.mult, op1=OP.add)
                nc.vector.scalar_tensor_tensor(d0x[:], rbx[:], 256.0, p1x[:],
                                               op0=OP.mult, op1=OP.add)
                i0y, i0x = T("i0y"), T("i0x")
                nc.vector.tensor_tensor(i0y[:], hfly[:], slotB[:], op=OP.add)
                nc.vector.tensor_tensor(i0x[:], hflx[:], slotBX[:], op=OP.add)
                uy, ux = T("uy"), T("ux")
                nc.vector.tensor_tensor(uy[:], i0y[:], my[:], op=OP.mult)
                nc.vector.tensor_tensor(ux[:], i0x[:], mx[:], op=OP.mult)
                tmy, tmx = T("tmy"), T("tmx")
                nc.vector.tensor_scalar(tmy[:], my[:], 2.0, -1.0,
                                        op0=OP.mult, op1=OP.add)
                nc.vector.tensor_scalar(tmx[:], mx[:], 2.0, -1.0,
                                        op0=OP.mult, op1=OP.add)
                i1y, i1x = T("i1y"), T("i1x")
                nc.vector.tensor_tensor(i1y[:], uy[:], tmy[:], op=OP.add)
                nc.vector.tensor_tensor(i1x[:], ux[:], tmx[:], op=OP.add)

                ydatB = prep.tile([128, 2 * NG, 2 * G], U16, tag="ydatB")
                yidxB = prep.tile([128, 2 * NG, 2 * G], I16, tag="yidxB")
                xdatB = prep.tile([128, 2 * NG, 2 * G], U16, tag="xdatB")
                xidxB = prep.tile([128, 2 * NG, 2 * G], I16, tag="xidxB")
                nc.vector.tensor_scalar(ydatB[:, :, 0:G], d0y[:], 1.0, None,
                                        op0=OP.mult)
                nc.vector.tensor_scalar(xdatB[:, :, 0:G], d0x[:], 1.0, None,
                                        op0=OP.mult)
                nc.vector.tensor_scalar(ydatB[:, :, G:2 * G],
                                        bfy[:, W2:2 * W2], 1.0, None,
                                        op0=OP.mult)
                nc.vector.tensor_scalar(xdatB[:, :, G:2 * G],
                                        bfx[:, W2:2 * W2], 1.0, None,
                                        op0=OP.mult)
                nc.vector.tensor_scalar(yidxB[:, :, 0:G], i0y[:], 1.0, None,
                                        op0=OP.mult)
                nc.vector.tensor_scalar(xidxB[:, :, 0:G], i0x[:], 1.0, None,
                                        op0=OP.mult)
                nc.vector.tensor_scalar(yidxB[:, :, G:2 * G], i1y[:], 1.0,
                                        None, op0=OP.mult)
                nc.vector.tensor_scalar(xidxB[:, :, G:2 * G], i1x[:], 1.0,
                                        None, op0=OP.mult)

                # dense-x constants (full width; sliced per column later).
                # prep pool: read by ACT/DVE dense ops through the whole
                # body, so they need cross-body double buffering.
                ncxB = cxp1B = omcxB = None
                if any(('A' in a) or ('H' in a) for a in X_ASSIGN):
                    ncxB = prep.tile([128, W2], F32, tag="ncxB", name="ncxB")
                    nc.vector.tensor_scalar(ncxB[:], cxoB[:], -1.0, None,
                                            op0=OP.mult)
                if any(('D' in a) or ('J' in a) for a in X_ASSIGN):
                    cxp1B = prep.tile([128, W2], F32, tag="cxp1B",
                                      name="cxp1B")
                    nc.vector.tensor_scalar(cxp1B[:], cxoB[:], 1.0, None,
                                            op0=OP.add)
                    omcxB = prep.tile([128, W2], F32, tag="omcxB",
                                      name="omcxB")
                    nc.vector.tensor_scalar(omcxB[:], cxoB[:], -1.0, 1.0,
                                            op0=OP.mult, op1=OP.add)

                # ---- group loop: produce profiles, DR-matmul ----
                for g in range(NG):
                    for b in range(IMGS):
                        kind = X_ASSIGN[b][g]
                        gi = NG * b + g          # merged group index
                        xd8 = dstp.tile([128, G, 256], F8, tag=f"xd{b}",
                                        name=f"xd8_{b}")
                        if kind == "G":
                            nc.gpsimd.local_scatter(
                                xd8[:, :, :].bitcast(U16),
                                xdatB[:, gi, :], xidxB[:, gi, :],
                                channels=128, num_elems=NCELL,
                                num_idxs=2 * G)
                        elif kind in "HJ":
                            # chunks [HSPLIT,8) via sub-tile scatter (idx
                            # window [HSPLIT:16]; dense chunks' idx1 entries
                            # are negative sentinels, ignored by the ucode)
                            nc.gpsimd.local_scatter(
                                xd8[:, HSPLIT:G, :].bitcast(U16),
                                xdatB[:, gi, HSPLIT:16],
                                xidxB[:, gi, HSPLIT:16],
                                channels=128,
                                num_elems=(G - HSPLIT) * 128,
                                num_idxs=16 - HSPLIT)
                            for s in range(HSPLIT):
                                col = BODY_C * b + G * g + s
                                if kind == "H":
                                    u2 = tmp.tile([128, 256], F16,
                                                  tag=f"hx_u{s % 2}",
                                                  name="hx_u")
                                    nc.scalar.activation(
                                        u2[:], iota[:], ACT.Abs,
                                        bias=ncxB[:, col:col + 1], scale=1.0)
                                    nc.scalar.activation(
                                        xd8[:, s, :], u2[:], ACT.Relu,
                                        bias=1.0, scale=-1.0)
                                else:
                                    ee = tmp.tile([128, 256], BF16,
                                                  tag="jx_e", name="jx_e")
                                    nc.vector.tensor_scalar(
                                        ee[:], iotab[:], -1.0,
                                        cxp1B[:, col:col + 1],
                                        op0=OP.mult, op1=OP.add)
                                    ff = tmp.tile([128, 256], BF16,
                                                  tag="jx_f", name="jx_f")
                                    nc.vector.tensor_scalar(
                                        ff[:], iotab[:],
                                        omcxB[:, col:col + 1], None,
                                        op0=OP.add)
                                    mm = tmp.tile([128, 256], BF16,
                                                  tag="jx_m", name="jx_m")
                                    nc.vector.tensor_tensor(mm[:], ee[:],
                                                            ff[:], op=OP.min)
                                    nc.vector.tensor_scalar(
                                        xd8[:, s, :], mm[:], 1.0, 0.0,
                                        op0=OP.mult, op1=OP.max)
                        elif kind == "A":
                            for s in range(G):
                                col = BODY_C * b + G * g + s
                                u2 = tmp.tile([128, 256], F16,
                                              tag=f"ax_u{s % 2}",
                                              name="ax_u")
                                nc.scalar.activation(
                                    u2[:], iota[:], ACT.Abs,
                                    bias=ncxB[:, col:col + 1], scale=1.0)
                                nc.scalar.activation(
                                    xd8[:, s, :], u2[:], ACT.Relu,
                                    bias=1.0, scale=-1.0)
                        else:
                            for s in range(G):
                                col = BODY_C * b + G * g + s
                                ee = tmp.tile([128, 256], BF16, tag="dx_e",
                                              name="dx_e")
                                nc.vector.tensor_scalar(
                                    ee[:], iotab[:], -1.0,
                                    cxp1B[:, col:col + 1],
                                    op0=OP.mult, op1=OP.add)
                                ff = tmp.tile([128, 256], BF16, tag="dx_f",
                                              name="dx_f")
                                nc.vector.tensor_scalar(
                                    ff[:], iotab[:],
                                    omcxB[:, col:col + 1], None, op0=OP.add)
                                mm = tmp.tile([128, 256], BF16, tag="dx_m",
                                              name="dx_m")
                                nc.vector.tensor_tensor(mm[:], ee[:], ff[:],
                                                        op=OP.min)
                                nc.vector.tensor_scalar(
                                    xd8[:, s, :], mm[:], 1.0, 0.0,
                                    op0=OP.mult, op1=OP.max)
                        # y always GPSIMD pair-scatter
                        yd8 = dstp.tile([128, G, 256], F8, tag=f"yd{b}",
                                        name=f"yd8_{b}")
                        nc.gpsimd.local_scatter(
                            yd8[:, :, :].bitcast(U16),
                            ydatB[:, gi, :], yidxB[:, gi, :],
                            channels=128, num_elems=NCELL, num_idxs=2 * G)
                        # DoubleRow matmuls: 2 chunks per instruction
                        for pr in range(G // 2):
                            rhs = xd8[:, 2 * pr:2 * pr + 2, :]
                            for h in range(2):
                                lhsT = yd8[:, 2 * pr:2 * pr + 2,
                                           128 * h:128 * (h + 1)]
                                nc.tensor.matmul(acc[b][h][:], lhsT, rhs,
                                                 perf_mode=PM.DoubleRow,
                                                 start=False, stop=False,
                                                 skip_group_check=True)

            UNROLL = 70
            with tc.For_i(0, N_ITER // UNROLL, 1) as it:
                for u in range(UNROLL):
                    emit_body(it * (BODY_C * UNROLL) + u * BODY_C)

            for b in range(IMGS):
                for h in range(2):
                    nc.tensor.matmul(acc[b][h][:], zero16[:, 0:128],
                                     zero16[:], start=False, stop=True,
                                     skip_group_check=True)

            # ---------------- epilogue: conv+FFT+CTF+iFFT ----------------
            def mm_pair(out_ps, lT, rhs_tiles, extra=None, first=True):
                ops = []
                for kc in range(2):
                    ops.append((lT[kc], rhs_tiles[kc]))
                if extra is not None:
                    lT2, rhs2 = extra
                    for kc in range(2):
                        ops.append((lT2[kc], rhs2[kc]))
                for j, (lt, rh) in enumerate(ops):
                    nc.tensor.matmul(out_ps[:], lt, rh,
                                     start=(first and j == 0),
                                     stop=(j == len(ops) - 1))

            def transpose_mat(src_tiles, tag):
                dst = [work.tile([128, 256], F32, tag=f"{tag}{m}",
                                 name=f"tr_{tag}_{m}")
                       for m in range(2)]
                for a in range(2):
                    for bcol in range(2):
                        pt = eps.tile([128, 128], F32, tag="ep")
                        nc.tensor.transpose(
                            pt[:], src_tiles[a][:, 128 * bcol:128 * (bcol + 1)],
                            ident[:])
                        nc.vector.tensor_copy(
                            dst[bcol][:, 128 * a:128 * (a + 1)], pt[:])
                return dst

            def cmul_stage(lr, li, nli, rhs_r, rhs_i, tag):
                outr, outi = [], []
                for m in range(2):
                    lrm = [lr[kc][:, 128 * m:128 * (m + 1)] for kc in range(2)]
                    lim = [li[kc][:, 128 * m:128 * (m + 1)] for kc in range(2)]
                    nlim = [nli[kc][:, 128 * m:128 * (m + 1)] for kc in range(2)]
                    pr = eps.tile([128, 256], F32, tag="ep")
                    mm_pair(pr, lrm, rhs_r, extra=(nlim, rhs_i))
                    tr = work.tile([128, 256], F32, tag=f"{tag}r{m}")
                    nc.vector.tensor_copy(tr[:], pr[:])
                    outr.append(tr)
                    pi = eps.tile([128, 256], F32, tag="ep")
                    mm_pair(pi, lrm, rhs_i, extra=(lim, rhs_r))
                    ti = work.tile([128, 256], F32, tag=f"{tag}i{m}")
                    nc.vector.tensor_copy(ti[:], pi[:])
                    outi.append(ti)
                return outr, outi

            for b in range(IMGS):
                img_sb = [work.tile([128, 256], F32, tag=f"img{h}",
                                    name=f"img_sb_{h}")
                          for h in range(2)]
                for h in range(2):
                    nc.vector.tensor_copy(img_sb[h][:], acc[b][h][:])
                Ur, Ui = [], []
                for m in range(2):
                    for part, lst in (("r", Ur), ("i", Ui)):
                        mat = mats["ATr" if part == "r" else "ATi"]
                        ps = eps.tile([128, 256], F32, tag="ep")
                        mm_pair(ps, [mat[kc][:, 128 * m:128 * (m + 1)]
                                     for kc in range(2)], img_sb)
                        t = work.tile([128, 256], F32, tag=f"U{part}{m}")
                        nc.vector.tensor_copy(t[:], ps[:])
                        lst.append(t)
                UTr = transpose_mat(Ur, "UTr")
                UTi = transpose_mat(Ui, "UTi")
                STr, STi = cmul_stage(mats["BrT"], mats["BiT"], mats["nBiT"],
                                      UTr, UTi, "ST")
                Spr, Spi = [], []
                for m in range(2):
                    tr = work.tile([128, 256], F32, tag=f"Spr{m}")
                    nc.vector.tensor_tensor(tr[:], STr[m][:], ctfs[b][m][:],
                                            op=OP.mult)
                    Spr.append(tr)
                    ti = work.tile([128, 256], F32, tag=f"Spi{m}")
                    nc.vector.tensor_tensor(ti[:], STi[m][:], ctfs[b][m][:],
                                            op=OP.mult)
                    Spi.append(ti)
                SpTr = transpose_mat(Spr, "SpTr")
                SpTi = transpose_mat(Spi, "SpTi")
                Wr, Wi = cmul_stage(mats["IFrT"], mats["IFiT"], mats["nIFiT"],
                                    SpTr, SpTi, "W")
                WTr = transpose_mat(Wr, "WTr")
                WTi = transpose_mat(Wi, "WTi")
                for m in range(2):
                    po = eps.tile([128, 256], F32, tag="ep")
                    mm_pair(po, [mats["EXrT"][kc][:, 128 * m:128 * (m + 1)]
                                 for kc in range(2)], WTr,
                            extra=([mats["nEXiT"][kc][:, 128 * m:128 * (m + 1)]
                                    for kc in range(2)], WTi))
                    ot = work.tile([128, 256], F32, tag=f"outT{m}")
                    nc.vector.tensor_copy(ot[:], po[:])
                    nc.sync.dma_start(out_d[b, 128 * m:128 * (m + 1), :], ot[:])
    nc.finalize()
    return nc


_NC_CACHE = None
_TRACE = False
_TMPDIR = None
_LAST_RES = None


def _get_nc():
    global _NC_CACHE
    if _NC_CACHE is None:
        _NC_CACHE = _build_nc()
    return _NC_CACHE


# ---------------------------------------------------------------- host entry
def kernel(alignment, shifts, coords, values, ctf):
    alignment = np.asarray(alignment, np.float32)
    shifts = np.asarray(shifts, np.float32)
    coords = np.asarray(coords, np.float32)
    values = np.asarray(values, np.float32)
    ctf = np.asarray(ctf, np.float32)

    # pad points; pad coords with a copy of point 0 (in range), v=0
    cpad = np.empty((NPAD, 3), np.float32)
    cpad[:N_PTS] = coords
    cpad[N_PTS:] = coords[0]
    vpad = np.zeros((NPAD,), np.float32)
    vpad[:N_PTS] = values
    fx = np.ascontiguousarray(cpad[:, 0].reshape(128, NCHUNK))
    fy = np.ascontiguousarray(cpad[:, 1].reshape(128, NCHUNK))
    fz = np.ascontiguousarray(cpad[:, 2].reshape(128, NCHUNK))
    fv = np.ascontiguousarray(vpad.reshape(128, NCHUNK))

    mats = _build_mats()
    iota = np.ascontiguousarray(
        np.arange(256, dtype=np.float32)[None, :].repeat(128, axis=0))
    # pair-cell slot offsets: 128*(chunk%G), doubled for both images
    slot1 = (128.0 * (np.arange(BODY_C) % G)).astype(np.float32)
    slotB = np.ascontiguousarray(
        np.concatenate([slot1, slot1])[None, :].repeat(128, axis=0))
    # x slots: hybrid groups scatter only chunks 4..7 into a half-size
    # sub-tile (slot base shifted by -512); dense chunks get a large
    # negative sentinel so their idx entries are ignored
    sx = []
    for b in range(IMGS):
        for c in range(BODY_C):
            g, sch = c // G, c % G
            kind = X_ASSIGN[b][g]
            if kind in "HJ":
                sx.append(128.0 * (sch - HSPLIT) if sch >= HSPLIT
                          else -100000.0)
            else:
                sx.append(128.0 * sch)
    slotBX = np.ascontiguousarray(
        np.asarray(sx, np.float32)[None, :].repeat(128, axis=0))

    in_maps = []
    for c in range(N_CORES):
        b0 = IMGS * c
        sc = np.zeros((128, IMGS * 16), np.float32)
        for b in range(IMGS):
            al = alignment[b0 + b]
            o = 16 * b
            sc[:, o + C_A:o + C_A + 3] = al[0:3]
            sc[:, o + C_B:o + C_B + 3] = al[3:6]
            sc[:, o + C_CY] = 128.0 - shifts[b0 + b, 1]
            sc[:, o + C_CX] = 128.0 - shifts[b0 + b, 0]
        ctfT = np.zeros((IMGS, 256, 256), np.float32)
        ctfT[:, :KX, :] = np.transpose(ctf[b0:b0 + IMGS], (0, 2, 1))
        m = {"xt": fx, "yt": fy, "zt": fz, "vt": fv,
             "sc": sc, "slotB": slotB, "slotBX": slotBX,
             "iota": iota, "ctfT": ctfT,
             "ident": mats["ident"]}
        for name in MAT_NAMES:
            m[name] = mats[name]
        in_maps.append(m)

    nc = _get_nc()
    res = run_bass_kernel_spmd(nc, in_maps, list(range(N_CORES)),
                               trace=_TRACE, tmpdir=_TMPDIR)
    global _LAST_RES
    _LAST_RES = res
    out = np.empty((B_FULL, 256, 256), np.float32)
    for c in range(N_CORES):
        o = res.results[c]["out"]          # (2, 256, 256) x-major
        for b in range(IMGS):
            out[IMGS * c + b] = o[b].T
    return out


if __name__ == "__main__":
    d = np.load("/root/problem/work/ref_cache.npz")
    ins = {k: d[k] for k in ["alignment", "shifts", "coords", "values", "ctf"]}
    o = kernel(**ins)
    ref = d["ref"]
    err = np.abs(o - ref).max() / np.abs(ref).max()
    print("rel err:", err)


# revision 23
# speedup vs baseline: 1.0058x; 1.0058x over previous
"""Trainium2 Bass kernel for nn_Decoder (scatter + gaussian conv + CTF filter).

Self-contained: hardcodes shapes/sharding for
  alignment (16,6), shifts (16,2), coords (500000,3), values (500000,),
  ctf (16,256,129) -> out (16,256,256) float32, 8 NeuronCores.

Sharding: pure data-parallel over the batch; each core handles 2 images.
Per core:
  - scatter: per 128-point chunk build both bilinear profile rows in fp8e4
    and accumulate the 256x256 image in PSUM with DoubleRow PE matmuls
    (2 chunks per instruction, both operands fp8e4 -> no power throttle).
    Both profiles are built as fp8 PAIRS packed in uint16 cells (the two
    bilinear taps are always adjacent: floor(c), floor(c)+1), mostly by
    GPSIMD local_scatter whose cost is the dst zero-fill (halved by the
    pair packing); overflow x-groups/half-groups are built densely by ACT
    (Abs then Relu, table preloaded) and DVE (tensor_scalar hat via
    min((c+1)-i, i+(1-c)) clamped by op1=max).  Values v are folded into
    the y-profile so x-profiles are unweighted.  All index/data prep runs
    on DVE as width-112 chains (both images merged into one tile);
    dtype converts use tensor_scalar (ALU speed), never tensor_copy CAST
    (~7ns/elem) and never stride-2 narrow writes (RMW, ~50ns/elem).
    The whole loop is fully unrolled: body boundaries cost ~6us each in
    pipeline drains otherwise.  Engine balance at steady state: GPSIMD
    ~95%, ACT ~100%, DVE ~91%, PE-array ~70% effective.
  - conv+FFT+CTF+iFFT: gaussian conv folded into precomputed DFT
    matrices; the whole linear chain is fp32 matmuls + PE transposes.
"""
import sys
if '/opt/trn_rl_repo' not in sys.path:
    sys.path.insert(0, '/opt/trn_rl_repo')

import numpy as np
import concourse.bass as bass
import concourse.bacc as bacc
import concourse.mybir as mybir
from concourse.tile import TileContext
from concourse.bass_utils import run_bass_kernel_spmd

F8 = mybir.dt.float8e4
F16 = mybir.dt.float16
BF16 = mybir.dt.bfloat16
F32 = mybir.dt.float32
U8 = mybir.dt.uint8
U16 = mybir.dt.uint16
I16 = mybir.dt.int16
I32 = mybir.dt.int32
OP = mybir.AluOpType
ACT = mybir.ActivationFunctionType
PM = mybir.MatmulPerfMode

XSIZE = 256
KX = 129
N_PTS = 500000
B_FULL = 16
N_CORES = 8
IMGS = 2                    # images per core
NCHUNK = 3920               # point chunks per image (128 pts each), padded
NPAD = NCHUNK * 128         # 501760 padded points
G = 8                       # chunks per group / dst tile
NG = 7                      # groups per body per image
BODY_C = G * NG             # 56 chunks per For_i body
N_ITER = NCHUNK // BODY_C   # 70
W2 = IMGS * BODY_C          # 112: merged-image prep width
NCELL = G * 128             # 1024 uint16 pair-cells per dst
HSPLIT = 4                  # hybrid groups: chunks [0,HSPLIT) dense, rest GPS

# per-image x-profile group assignment (group indices 0..6):
#   'D' = DVE dense, 'A' = ACT dense, 'G' = GPSIMD pair-scatter
X_ASSIGN = [
    "AJGGGGG",   # image 0: 1 ACT + half-DVE hybrid + 5 GPS
    "AAHGGGG",   # image 1: 2 ACT + half-ACT hybrid + 4 GPS
]


# ---------------------------------------------------------------- host mats
def _build_mats():
    n = XSIZE
    y = np.arange(n)
    ax = np.arange(5, dtype=np.float64) - 2.0
    g = np.exp(-(ax ** 2) / 2.0)
    gn = g / g.sum()
    Gm = np.zeros((n, n))
    for d in range(-2, 3):
        idx = np.arange(max(0, -d), min(n, n - d))
        Gm[idx, idx + d] = gn[d + 2]
    F = np.exp(-2j * np.pi * np.outer(y, y) / n)
    A = F @ Gm                                               # (256,256)
    Bh = np.exp(-2j * np.pi * np.outer(np.arange(KX), y) / n) @ Gm
    Bm = np.zeros((n, n), complex)
    Bm[:KX] = Bh                                             # kx zero-padded
    IFy = np.exp(+2j * np.pi * np.outer(y, y) / n) / n
    c = np.ones(KX)
    c[1:-1] = 2.0
    EXh = (np.exp(+2j * np.pi * np.outer(y, np.arange(KX)) / n) * c[None, :]) / n
    EX = np.zeros((n, n), complex)
    EX[:, :KX] = EXh

    def lhsT(M):  # (256,256) -> transposed, chunked (2,128,256) f32
        t = np.ascontiguousarray(M.T.reshape(2, 128, 256))
        return t.astype(np.float32)

    mats = {
        "ATr": lhsT(A.real), "ATi": lhsT(A.imag),
        "BrT": lhsT(Bm.real), "BiT": lhsT(Bm.imag), "nBiT": lhsT(-Bm.imag),
        "IFrT": lhsT(IFy.real), "IFiT": lhsT(IFy.imag), "nIFiT": lhsT(-IFy.imag),
        "EXrT": lhsT(EX.real), "nEXiT": lhsT(-EX.imag),
        "ident": np.eye(128, dtype=np.float32),
    }
    return mats


MAT_NAMES = ["ATr", "ATi", "BrT", "BiT", "nBiT", "IFrT", "IFiT", "nIFiT",
             "EXrT", "nEXiT"]

# sc columns (per image, 16 cols): 0-2 x row coeffs, 3-5 y row coeffs,
# 6 y const (128 - sy), 7 x const (128 - sx)
C_A, C_B, C_CY, C_CX = 0, 3, 6, 7


# ---------------------------------------------------------------- bass build
def _build_nc():
    nc = bacc.Bacc()
    xt_in = nc.declare_dram_parameter("xt", [128, NCHUNK], F32, isOutput=False)
    yt_in = nc.declare_dram_parameter("yt", [128, NCHUNK], F32, isOutput=False)
    zt_in = nc.declare_dram_parameter("zt", [128, NCHUNK], F32, isOutput=False)
    vt_in = nc.declare_dram_parameter("vt", [128, NCHUNK], F32, isOutput=False)
    sc_in = nc.declare_dram_parameter("sc", [128, IMGS * 16], F32,
                                      isOutput=False)
    slot_in = nc.declare_dram_parameter("slotB", [128, W2], F32,
                                        isOutput=False)
    slotx_in = nc.declare_dram_parameter("slotBX", [128, W2], F32,
                                         isOutput=False)
    iota_in = nc.declare_dram_parameter("iota", [128, 256], F32,
                                        isOutput=False)
    ctf_in = nc.declare_dram_parameter("ctfT", [IMGS, 256, 256], F32,
                                       isOutput=False)
    mat_in = {m: nc.declare_dram_parameter(m, [2, 128, 256], F32,
                                           isOutput=False)
              for m in MAT_NAMES}
    id_in = nc.declare_dram_parameter("ident", [128, 128], F32, isOutput=False)
    out_d = nc.declare_dram_parameter("out", [IMGS, 256, 256], F32,
                                      isOutput=True)

    with TileContext(nc) as tc:
        with tc.tile_pool(name="inp", bufs=1) as inp, \
             tc.tile_pool(name="mat", bufs=1) as matp, \
             tc.tile_pool(name="tmp", bufs=1) as tmp, \
             tc.tile_pool(name="prep", bufs=3) as prep, \
             tc.tile_pool(name="dstp", bufs=6) as dstp, \
             tc.tile_pool(name="work", bufs=1) as work, \
             tc.tile_pool(name="accp", bufs=1, space="PSUM") as accp, \
             tc.tile_pool(name="eps", bufs=4, space="PSUM") as eps:

            # ---------------- load inputs ----------------
            xt = inp.tile([128, NCHUNK], F32)
            yt = inp.tile([128, NCHUNK], F32)
            zt = inp.tile([128, NCHUNK], F32)
            vt = inp.tile([128, NCHUNK], F32)
            nc.sync.dma_start(xt[:], xt_in[:])
            nc.sync.dma_start(yt[:], yt_in[:])
            nc.sync.dma_start(zt[:], zt_in[:])
            nc.sync.dma_start(vt[:], vt_in[:])

            sc = inp.tile([128, IMGS * 16], F32)
            nc.sync.dma_start(sc[:], sc_in[:])
            slotB = inp.tile([128, W2], F32)
            nc.sync.dma_start(slotB[:], slot_in[:])
            slotBX = inp.tile([128, W2], F32)
            nc.sync.dma_start(slotBX[:], slotx_in[:])
            halfB = inp.tile([128, W2], F32)
            nc.vector.memset(halfB[:], 0.5)
            # iota passes through Abs+Relu (exact on >=0 values) so the ACT
            # function table is loaded before the loop.
            iota_raw = inp.tile([128, 256], F32)
            nc.sync.dma_start(iota_raw[:], iota_in[:])
            iota = inp.tile([128, 256], F32)
            nc.scalar.activation(iota[:], iota_raw[:], ACT.Abs,
                                 bias=0.0, scale=1.0)
            nc.scalar.activation(iota[:], iota[:], ACT.Relu,
                                 bias=0.0, scale=1.0)
            iotab = inp.tile([128, 256], BF16)
            nc.vector.tensor_scalar(iotab[:], iota[:], 1.0, None, op0=OP.mult)

            mats = {}
            for m in MAT_NAMES:
                t0 = matp.tile([128, 256], F32, tag=f"{m}0")
                t1 = matp.tile([128, 256], F32, tag=f"{m}1")
                nc.sync.dma_start(t0[:], mat_in[m][0])
                nc.sync.dma_start(t1[:], mat_in[m][1])
                mats[m] = (t0, t1)
            ident = matp.tile([128, 128], F32)
            nc.sync.dma_start(ident[:], id_in[:])
            ctfs = []
            for b in range(IMGS):
                c0 = matp.tile([128, 256], F32, tag=f"ctf{b}0")
                c1 = matp.tile([128, 256], F32, tag=f"ctf{b}1")
                nc.sync.dma_start(c0[:], ctf_in[b, 0:128, :])
                nc.sync.dma_start(c1[:], ctf_in[b, 128:256, :])
                ctfs.append((c0, c1))

            zero16 = inp.tile([128, 256], F16)
            nc.vector.memset(zero16[:], 0.0)

            # GPSIMD local_scatter ucode preload
            dum_idx = inp.tile([128, 2], I16)
            nc.vector.memset(dum_idx[:], -1.0)
            dum_dat = inp.tile([128, 2], U16)
            nc.vector.memset(dum_dat[:], 0.0)
            dum_dst = inp.tile([128, 2], U16)
            nc.gpsimd.local_scatter(dum_dst[:], dum_dat[:], dum_idx[:],
                                    channels=128, num_elems=2, num_idxs=2)

            # ---------------- PSUM accumulators ----------------
            acc = [[accp.tile([128, 256], F32, tag=f"acc{b}{h}",
                               name=f"acc_{b}_{h}")
                    for h in range(2)] for b in range(IMGS)]
            for b in range(IMGS):
                for h in range(2):
                    nc.tensor.matmul(acc[b][h][:], zero16[:, 0:128],
                                     zero16[:], start=True, stop=False,
                                     skip_group_check=True)

            # ---------------- main scatter loop ----------------
            def proj(dst, base, cc0, cc1, cc2, last_scalar):
                """dst = xt*c0 + yt*c1 + zt*c2 + scalar over BODY_C chunks."""
                t0 = tmp.tile([128, BODY_C], F32, tag="p_t0")
                nc.vector.tensor_scalar(
                    t0[:], xt[:, bass.DynSlice(base, BODY_C)], cc0,
                    last_scalar, op0=OP.mult, op1=OP.add)
                t1 = tmp.tile([128, BODY_C], F32, tag="p_t1")
                nc.vector.scalar_tensor_tensor(
                    t1[:], yt[:, bass.DynSlice(base, BODY_C)], cc1, t0[:],
                    op0=OP.mult, op1=OP.add)
                nc.vector.scalar_tensor_tensor(
                    dst, zt[:, bass.DynSlice(base, BODY_C)], cc2, t1[:],
                    op0=OP.mult, op1=OP.add)

            def emit_body(base):
                vcurB = prep.tile([128, W2], F32, tag="vcurB")
                nc.vector.tensor_scalar(vcurB[:, 0:BODY_C],
                                        vt[:, bass.DynSlice(base, BODY_C)],
                                        1.0, None, op0=OP.mult)
                nc.vector.tensor_scalar(vcurB[:, BODY_C:W2],
                                        vt[:, bass.DynSlice(base, BODY_C)],
                                        1.0, None, op0=OP.mult)

                cyoB = tmp.tile([128, W2], F32, tag="cyoB")
                cxoB = tmp.tile([128, W2], F32, tag="cxoB")
                for b in range(IMGS):
                    o = 16 * b
                    sl = slice(BODY_C * b, BODY_C * (b + 1))
                    proj(cxoB[:, sl], base, sc[:, o + C_A:o + C_A + 1],
                         sc[:, o + C_A + 1:o + C_A + 2],
                         sc[:, o + C_A + 2:o + C_A + 3],
                         sc[:, o + C_CX:o + C_CX + 1])
                    proj(cyoB[:, sl], base, sc[:, o + C_B:o + C_B + 1],
                         sc[:, o + C_B + 1:o + C_B + 2],
                         sc[:, o + C_B + 2:o + C_B + 3],
                         sc[:, o + C_CY:o + C_CY + 1])

                # ---- interleaved y/x pair-build chains (width 112) ----
                def T(nm, dt=F32, w=W2):
                    return tmp.tile([128, w], dt, tag=nm, name=nm)

                hcy, hcx = T("hcy"), T("hcx")
                nc.vector.tensor_scalar(hcy[:], cyoB[:], 0.5, None, op0=OP.mult)
                nc.vector.tensor_scalar(hcx[:], cxoB[:], 0.5, None, op0=OP.mult)
                iiy, iix = T("iiy", I32), T("iix", I32)
                nc.vector.tensor_scalar(iiy[:], hcy[:], 1.0, None, op0=OP.mult)
                nc.vector.tensor_scalar(iix[:], hcx[:], 1.0, None, op0=OP.mult)
                ddy, ddx = T("ddy"), T("ddx")
                nc.vector.tensor_scalar(ddy[:], iiy[:], 1.0, None, op0=OP.mult)
                nc.vector.tensor_scalar(ddx[:], iix[:], 1.0, None, op0=OP.mult)
                gty, gtx = T("gty"), T("gtx")
                nc.vector.tensor_tensor(gty[:], ddy[:], hcy[:], op=OP.is_gt)
                nc.vector.tensor_tensor(gtx[:], ddx[:], hcx[:], op=OP.is_gt)
                hfly, hflx = T("hfly"), T("hflx")
                nc.vector.tensor_tensor(hfly[:], ddy[:], gty[:], op=OP.subtract)
                nc.vector.tensor_tensor(hflx[:], ddx[:], gtx[:], op=OP.subtract)
                ry, rx = T("ry"), T("rx")
                nc.vector.tensor_tensor(ry[:], hcy[:], hfly[:], op=OP.subtract)
                nc.vector.tensor_tensor(rx[:], hcx[:], hflx[:], op=OP.subtract)
                my, mx = T("my"), T("mx")
                nc.vector.tensor_tensor(my[:], ry[:], halfB[:], op=OP.is_ge)
                nc.vector.tensor_tensor(mx[:], rx[:], halfB[:], op=OP.is_ge)
                fyB, fxB = T("fyB"), T("fxB")
                nc.vector.scalar_tensor_tensor(fyB[:], ry[:], 2.0, my[:],
                                               op0=OP.mult, op1=OP.subtract)
                nc.vector.scalar_tensor_tensor(fxB[:], rx[:], 2.0, mx[:],
                                               op0=OP.mult, op1=OP.subtract)
                w1v = T("w1v")
                nc.vector.tensor_tensor(w1v[:], vcurB[:], fyB[:], op=OP.mult)
                w0x = T("w0x")
                nc.vector.tensor_scalar(w0x[:], fxB[:], -1.0, 1.0,
                                        op0=OP.mult, op1=OP.add)
                w0v = T("w0v")
                nc.vector.tensor_tensor(w0v[:], vcurB[:], w1v[:], op=OP.subtract)
                c8y = T("c8y", F8, 2 * W2)
                c8x = T("c8x", F8, 2 * W2)
                nc.vector.tensor_scalar(c8x[:, 0:W2], w0x[:], 1.0, None,
                                        op0=OP.mult)
                nc.vector.tensor_scalar(c8x[:, W2:2 * W2], fxB[:], 1.0, None,
                                        op0=OP.mult)
                nc.vector.tensor_scalar(c8y[:, 0:W2], w0v[:], 1.0, None,
                                        op0=OP.mult)
                nc.vector.tensor_scalar(c8y[:, W2:2 * W2], w1v[:], 1.0, None,
                                        op0=OP.mult)
                bfy, bfx = T("bfy", F32, 2 * W2), T("bfx", F32, 2 * W2)
                nc.vector.tensor_scalar(bfy[:], c8y[:].bitcast(U8), 1.0, None,
                                        op0=OP.mult)
                nc.vector.tensor_scalar(bfx[:], c8x[:].bitcast(U8), 1.0, None,
                                        op0=OP.mult)
                qvy, qvx = T("qvy"), T("qvx")
                nc.vector.tensor_scalar(qvy[:], my[:], 255.0, 1.0,
                                        op0=OP.mult, op1=OP.add)
                nc.vector.tensor_scalar(qvx[:], mx[:], 255.0, 1.0,
                                        op0=OP.mult, op1=OP.add)
                p1y, p1x = T("p1y"), T("p1x")
                nc.vector.tensor_tensor(p1y[:], bfy[:, 0:W2], qvy[:], op=OP.mult)
                nc.vector.tensor_tensor(p1x[:], bfx[:, 0:W2], qvx[:], op=OP.mult)
                eyt, ext = T("eyt"), T("ext")
                nc.vector.tensor_scalar(eyt[:], my[:], -1.0, 1.0,
                                        op0=OP.mult, op1=OP.add)
                nc.vector.tensor_scalar(ext[:], mx[:], -1.0, 1.0,
                                        op0=OP.mult, op1=OP.add)
                rby, rbx = T("rby"), T("rbx")
                nc.vector.tensor_tensor(rby[:], bfy[:, W2:2 * W2], eyt[:],
                                        op=OP.mult)
                nc.vector.tensor_tensor(rbx[:], bfx[:, W2:2 * W2], ext[:],
                                        op=OP.mult)
                d0y, d0x = T("d0y"), T("d0x")
                nc.vector.scalar_tensor_tensor(d0y[:], rby[:], 256.0, p1y[:],
                                               op0=OP.mult, op1=OP.add)
                nc.vector.scalar_tensor_tensor(d0x[:], rbx[:], 256.0, p1x[:],
                                               op0=OP.mult, op1=OP.add)
                i0y, i0x = T("i0y"), T("i0x")
                nc.vector.tensor_tensor(i0y[:], hfly[:], slotB[:], op=OP.add)
                nc.vector.tensor_tensor(i0x[:], hflx[:], slotBX[:], op=OP.add)
                uy, ux = T("uy"), T("ux")
                nc.vector.tensor_tensor(uy[:], i0y[:], my[:], op=OP.mult)
                nc.vector.tensor_tensor(ux[:], i0x[:], mx[:], op=OP.mult)
                tmy, tmx = T("tmy"), T("tmx")
                nc.vector.tensor_scalar(tmy[:], my[:], 2.0, -1.0,
                                        op0=OP.mult, op1=OP.add)
                nc.vector.tensor_scalar(tmx[:], mx[:], 2.0, -1.0,
                                        op0=OP.mult, op1=OP.add)
                i1y, i1x = T("i1y"), T("i1x")
                nc.vector.tensor_tensor(i1y[:], uy[:], tmy[:], op=OP.add)
                nc.vector.tensor_tensor(i1x[:], ux[:], tmx[:], op=OP.add)

                ydatB = prep.tile([128, 2 * NG, 2 * G], U16, tag="ydatB")
                yidxB = prep.tile([128, 2 * NG, 2 * G], I16, tag="yidxB")
                xdatB = prep.tile([128, 2 * NG, 2 * G], U16, tag="xdatB")
                xidxB = prep.tile([128, 2 * NG, 2 * G], I16, tag="xidxB")
                nc.vector.tensor_scalar(ydatB[:, :, 0:G], d0y[:], 1.0, None,
                                        op0=OP.mult)
                nc.vector.tensor_scalar(xdatB[:, :, 0:G], d0x[:], 1.0, None,
                                        op0=OP.mult)
                nc.vector.tensor_scalar(ydatB[:, :, G:2 * G],
                                        bfy[:, W2:2 * W2], 1.0, None,
                                        op0=OP.mult)
                nc.vector.tensor_scalar(xdatB[:, :, G:2 * G],
                                        bfx[:, W2:2 * W2], 1.0, None,
                                        op0=OP.mult)
                nc.vector.tensor_scalar(yidxB[:, :, 0:G], i0y[:], 1.0, None,
                                        op0=OP.mult)
                nc.vector.tensor_scalar(xidxB[:, :, 0:G], i0x[:], 1.0, None,
                                        op0=OP.mult)
                nc.vector.tensor_scalar(yidxB[:, :, G:2 * G], i1y[:], 1.0,
                                        None, op0=OP.mult)
                nc.vector.tensor_scalar(xidxB[:, :, G:2 * G], i1x[:], 1.0,
                                        None, op0=OP.mult)

                # dense-x constants (full width; sliced per column later).
                # prep pool: read by ACT/DVE dense ops through the whole
                # body, so they need cross-body double buffering.
                ncxB = cxp1B = omcxB = None
                if any(('A' in a) or ('H' in a) for a in X_ASSIGN):
                    ncxB = prep.tile([128, W2], F32, tag="ncxB", name="ncxB")
                    nc.vector.tensor_scalar(ncxB[:], cxoB[:], -1.0, None,
                                            op0=OP.mult)
                if any(('D' in a) or ('J' in a) for a in X_ASSIGN):
                    cxp1B = prep.tile([128, W2], F32, tag="cxp1B",
                                      name="cxp1B")
                    nc.vector.tensor_scalar(cxp1B[:], cxoB[:], 1.0, None,
                                            op0=OP.add)
                    omcxB = prep.tile([128, W2], F32, tag="omcxB",
                                      name="omcxB")
                    nc.vector.tensor_scalar(omcxB[:], cxoB[:], -1.0, 1.0,
                                            op0=OP.mult, op1=OP.add)

                # ---- group loop: produce profiles, DR-matmul ----
                for g in range(NG):
                    for b in range(IMGS):
                        kind = X_ASSIGN[b][g]
                        gi = NG * b + g          # merged group index
                        xd8 = dstp.tile([128, G, 256], F8, tag=f"xd{b}",
                                        name=f"xd8_{b}")
                        if kind == "G":
                            nc.gpsimd.local_scatter(
                                xd8[:, :, :].bitcast(U16),
                                xdatB[:, gi, :], xidxB[:, gi, :],
                                channels=128, num_elems=NCELL,
                                num_idxs=2 * G)
                        elif kind in "HJ":
                            # chunks [HSPLIT,8) via sub-tile scatter (idx
                            # window [HSPLIT:16]; dense chunks' idx1 entries
                            # are negative sentinels, ignored by the ucode)
                            nc.gpsimd.local_scatter(
                                xd8[:, HSPLIT:G, :].bitcast(U16),
                                xdatB[:, gi, HSPLIT:16],
                                xidxB[:, gi, HSPLIT:16],
                                channels=128,
                                num_elems=(G - HSPLIT) * 128,
                                num_idxs=16 - HSPLIT)
                            for s in range(HSPLIT):
                                col = BODY_C * b + G * g + s
                                if kind == "H":
                                    u2 = tmp.tile([128, 256], F16,
                                                  tag=f"hx_u{s % 2}",
                                                  name="hx_u")
                                    nc.scalar.activation(
                                        u2[:], iota[:], ACT.Abs,
                                        bias=ncxB[:, col:col + 1], scale=1.0)
                                    nc.scalar.activation(
                                        xd8[:, s, :], u2[:], ACT.Relu,
                                        bias=1.0, scale=-1.0)
                                else:
                                    ee = tmp.tile([128, 256], BF16,
                                                  tag="jx_e", name="jx_e")
                                    nc.vector.tensor_scalar(
                                        ee[:], iotab[:], -1.0,
                                        cxp1B[:, col:col + 1],
                                        op0=OP.mult, op1=OP.add)
                                    ff = tmp.tile([128, 256], BF16,
                                                  tag="jx_f", name="jx_f")
                                    nc.vector.tensor_scalar(
                                        ff[:], iotab[:],
                                        omcxB[:, col:col + 1], None,
                                        op0=OP.add)
                                    mm = tmp.tile([128, 256], BF16,
                                                  tag="jx_m", name="jx_m")
                                    nc.vector.tensor_tensor(mm[:], ee[:],
                                                            ff[:], op=OP.min)
                                    nc.vector.tensor_scalar(
                                        xd8[:, s, :], mm[:], 1.0, 0.0,
                                        op0=OP.mult, op1=OP.max)
                        elif kind == "A":
                            for s in range(G):
                                col = BODY_C * b + G * g + s
                                u2 = tmp.tile([128, 256], F16,
                                              tag=f"ax_u{s % 2}",
                                              name="ax_u")
                                nc.scalar.activation(
                                    u2[:], iota[:], ACT.Abs,
                                    bias=ncxB[:, col:col + 1], scale=1.0)
                                nc.scalar.activation(
                                    xd8[:, s, :], u2[:], ACT.Relu,
                                    bias=1.0, scale=-1.0)
                        else:
                            for s in range(G):
                                col = BODY_C * b + G * g + s
                                ee = tmp.tile([128, 256], BF16, tag="dx_e",
                                              name="dx_e")
                                nc.vector.tensor_scalar(
                                    ee[:], iotab[:], -1.0,
                                    cxp1B[:, col:col + 1],
                                    op0=OP.mult, op1=OP.add)
                                ff = tmp.tile([128, 256], BF16, tag="dx_f",
                                              name="dx_f")
                                nc.vector.tensor_scalar(
                                    ff[:], iotab[:],
                                    omcxB[:, col:col + 1], None, op0=OP.add)
                                mm = tmp.tile([128, 256], BF16, tag="dx_m",
                                              name="dx_m")
                                nc.vector.tensor_tensor(mm[:], ee[:], ff[:],
                                                        op=OP.min)
                                nc.vector.tensor_scalar(
                                    xd8[:, s, :], mm[:], 1.0, 0.0,
                                    op0=OP.mult, op1=OP.max)
                        # y always GPSIMD pair-scatter
                        yd8 = dstp.tile([128, G, 256], F8, tag=f"yd{b}",
                                        name=f"yd8_{b}")
                        nc.gpsimd.local_scatter(
                            yd8[:, :, :].bitcast(U16),
                            ydatB[:, gi, :], yidxB[:, gi, :],
                            channels=128, num_elems=NCELL, num_idxs=2 * G)
                        # DoubleRow matmuls: 2 chunks per instruction
                        for pr in range(G // 2):
                            rhs = xd8[:, 2 * pr:2 * pr + 2, :]
                            for h in range(2):
                                lhsT = yd8[:, 2 * pr:2 * pr + 2,
                                           128 * h:128 * (h + 1)]
                                nc.tensor.matmul(acc[b][h][:], lhsT, rhs,
                                                 perf_mode=PM.DoubleRow,
                                                 start=False, stop=False,
                                                 skip_group_check=True)

            UNROLL = 70
            with tc.For_i(0, N_ITER // UNROLL, 1) as it:
                for u in range(UNROLL):
                    emit_body(it * (BODY_C * UNROLL) + u * BODY_C)

            for b in range(IMGS):
                for h in range(2):
                    nc.tensor.matmul(acc[b][h][:], zero16[:, 0:128],
                                     zero16[:], start=False, stop=True,
                                     skip_group_check=True)

            # ---------------- epilogue: conv+FFT+CTF+iFFT ----------------
            def mm_pair(out_ps, lT, rhs_tiles, extra=None, first=True):
                ops = []
                for kc in range(2):
                    ops.append((lT[kc], rhs_tiles[kc]))
                if extra is not None:
                    lT2, rhs2 = extra
                    for kc in range(2):
                        ops.append((lT2[kc], rhs2[kc]))
                for j, (lt, rh) in enumerate(ops):
                    nc.tensor.matmul(out_ps[:], lt, rh,
                                     start=(first and j == 0),
                                     stop=(j == len(ops) - 1))

            def transpose_mat(src_tiles, tag):
                dst = [work.tile([128, 256], F32, tag=f"{tag}{m}",
                                 name=f"tr_{tag}_{m}")
                       for m in range(2)]
                for a in range(2):
                    for bcol in range(2):
                        pt = eps.tile([128, 128], F32, tag="ep")
                        nc.tensor.transpose(
                            pt[:], src_tiles[a][:, 128 * bcol:128 * (bcol + 1)],
                            ident[:])
                        nc.vector.tensor_copy(
                            dst[bcol][:, 128 * a:128 * (a + 1)], pt[:])
                return dst

            def cmul_stage(lr, li, nli, rhs_r, rhs_i, tag):
                outr, outi = [], []
                for m in range(2):
                    lrm = [lr[kc][:, 128 * m:128 * (m + 1)] for kc in range(2)]
                    lim = [li[kc][:, 128 * m:128 * (m + 1)] for kc in range(2)]
                    nlim = [nli[kc][:, 128 * m:128 * (m + 1)] for kc in range(2)]
                    pr = eps.tile([128, 256], F32, tag="ep")
                    mm_pair(pr, lrm, rhs_r, extra=(nlim, rhs_i))
                    tr = work.tile([128, 256], F32, tag=f"{tag}r{m}")
                    nc.vector.tensor_copy(tr[:], pr[:])
                    outr.append(tr)
                    pi = eps.tile([128, 256], F32, tag="ep")
                    mm_pair(pi, lrm, rhs_i, extra=(lim, rhs_r))
                    ti = work.tile([128, 256], F32, tag=f"{tag}i{m}")
                    nc.vector.tensor_copy(ti[:], pi[:])
                    outi.append(ti)
                return outr, outi

            for b in range(IMGS):
                img_sb = [work.tile([128, 256], F32, tag=f"img{h}",
                                    name=f"img_sb_{h}")
                          for h in range(2)]
                for h in range(2):
                    nc.vector.tensor_copy(img_sb[h][:], acc[b][h][:])
                Ur, Ui = [], []
                for m in range(2):
                    for part, lst in (("r", Ur), ("i", Ui)):
                        mat = mats["ATr" if part == "r" else "ATi"]
                        ps = eps.tile([128, 256], F32, tag="ep")
                        mm_pair(ps, [mat[kc][:, 128 * m:128 * (m + 1)]
                                     for kc in range(2)], img_sb)
                        t = work.tile([128, 256], F32, tag=f"U{part}{m}")
                        nc.vector.tensor_copy(t[:], ps[:])
                        lst.append(t)
                UTr = transpose_mat(Ur, "UTr")
                UTi = transpose_mat(Ui, "UTi")
                STr, STi = cmul_stage(mats["BrT"], mats["BiT"], mats["nBiT"],
                                      UTr, UTi, "ST")
                Spr, Spi = [], []
                for m in range(2):
                    tr = work.tile([128, 256], F32, tag=f"Spr{m}")
                    nc.vector.tensor_tensor(tr[:], STr[m][:], ctfs[b][m][:],
                                            op=OP.mult)
                    Spr.append(tr)
                    ti = work.tile([128, 256], F32, tag=f"Spi{m}")
                    nc.vector.tensor_tensor(ti[:], STi[m][:], ctfs[b][m][:],
                                            op=OP.mult)
                    Spi.append(ti)
                SpTr = transpose_mat(Spr, "SpTr")
                SpTi = transpose_mat(Spi, "SpTi")
                Wr, Wi = cmul_stage(mats["IFrT"], mats["IFiT"], mats["nIFiT"],
                                    SpTr, SpTi, "W")
                WTr = transpose_mat(Wr, "WTr")
                WTi = transpose_mat(Wi, "WTi")
                for m in range(2):
                    po = eps.tile([128, 256], F32, tag="ep")
                    mm_pair(po, [mats["EXrT"][kc][:, 128 * m:128 * (m + 1)]
                                 for kc in range(2)], WTr,
                            extra=([mats["nEXiT"][kc][:, 128 * m:128 * (m + 1)]
                                    for kc in range(2)], WTi))
                    ot = work.tile([128, 256], F32, tag=f"outT{m}")
                    nc.vector.tensor_copy(ot[:], po[:])
                    nc.sync.dma_start(out_d[b, 128 * m:128 * (m + 1), :], ot[:])
    nc.finalize()
    return nc


_NC_CACHE = None
_TRACE = False
_TMPDIR = None
_LAST_RES = None


def _get_nc():
    global _NC_CACHE
    if _NC_CACHE is None:
        _NC_CACHE = _build_nc()
    return _NC_CACHE


# ---------------------------------------------------------------- host entry
def kernel(alignment, shifts, coords, values, ctf):
    alignment = np.asarray(alignment, np.float32)
    shifts = np.asarray(shifts, np.float32)
    coords = np.asarray(coords, np.float32)
    values = np.asarray(values, np.float32)
    ctf = np.asarray(ctf, np.float32)

    # pad points; pad coords with a copy of point 0 (in range), v=0
    cpad = np.empty((NPAD, 3), np.float32)
    cpad[:N_PTS] = coords
    cpad[N_PTS:] = coords[0]
    vpad = np.zeros((NPAD,), np.float32)
    vpad[:N_PTS] = values
    fx = np.ascontiguousarray(cpad[:, 0].reshape(128, NCHUNK))
    fy = np.ascontiguousarray(cpad[:, 1].reshape(128, NCHUNK))
    fz = np.ascontiguousarray(cpad[:, 2].reshape(128, NCHUNK))
    fv = np.ascontiguousarray(vpad.reshape(128, NCHUNK))

    mats = _build_mats()
    iota = np.ascontiguousarray(
        np.arange(256, dtype=np.float32)[None, :].repeat(128, axis=0))
    # pair-cell slot offsets: 128*(chunk%G), doubled for both images
    slot1 = (128.0 * (np.arange(BODY_C) % G)).astype(np.float32)
    slotB = np.ascontiguousarray(
        np.concatenate([slot1, slot1])[None, :].repeat(128, axis=0))
    # x slots: hybrid groups scatter only chunks 4..7 into a half-size
    # sub-tile (slot base shifted by -512); dense chunks get a large
    # negative sentinel so their idx entries are ignored
    sx = []
    for b in range(IMGS):
        for c in range(BODY_C):
            g, sch = c // G, c % G
            kind = X_ASSIGN[b][g]
            if kind in "HJ":
                sx.append(128.0 * (sch - HSPLIT) if sch >= HSPLIT
                          else -100000.0)
            else:
                sx.append(128.0 * sch)
    slotBX = np.ascontiguousarray(
        np.asarray(sx, np.float32)[None, :].repeat(128, axis=0))

    in_maps = []
    for c in range(N_CORES):
        b0 = IMGS * c
        sc = np.zeros((128, IMGS * 16), np.float32)
        for b in range(IMGS):
            al = alignment[b0 + b]
            o = 16 * b
            sc[:, o + C_A:o + C_A + 3] = al[0:3]
            sc[:, o + C_B:o + C_B + 3] = al[3:6]
            sc[:, o + C_CY] = 128.0 - shifts[b0 + b, 1]
            sc[:, o + C_CX] = 128.0 - shifts[b0 + b, 0]
        ctfT = np.zeros((IMGS, 256, 256), np.float32)
        ctfT[:, :KX, :] = np.transpose(ctf[b0:b0 + IMGS], (0, 2, 1))
        m = {"xt": fx, "yt": fy, "zt": fz, "vt": fv,
             "sc": sc, "slotB": slotB, "slotBX": slotBX,
             "iota": iota, "ctfT": ctfT,
             "ident": mats["ident"]}
        for name in MAT_NAMES:
            m[name] = mats[name]
        in_maps.append(m)

    nc = _get_nc()
    res = run_bass_kernel_spmd(nc, in_maps, list(range(N_CORES)),
                               trace=_TRACE, tmpdir=_TMPDIR)
    global _LAST_RES
    _LAST_RES = res
    out = np.empty((B_FULL, 256, 256), np.float32)
    for c in range(N_CORES):
        o = res.results[c]["out"]          # (2, 256, 256) x-major
        for b in range(IMGS):
            out[IMGS * c + b] = o[b].T
    return out


if __name__ == "__main__":
    d = np.load("/root/problem/work/ref_cache.npz")
    ins = {k: d[k] for k in ["alignment", "shifts", "coords", "values", "ctf"]}
    o = kernel(**ins)
    ref = d["ref"]
    err = np.abs(o - ref).max() / np.abs(ref).max()
    print("rel err:", err)
